# revision 34
# baseline (speedup 1.0000x reference)
"""Trainium2 Bass kernel for nn_Lookback: causal running-mean over T.

out[b, t, c] = (1/(t+1)) * sum_{s<=t} x[b, s, c],  x: [8, 4096, 1024] fp32.

Sharding: data-parallel over batch B — core b handles batch b.
All I/O is fp16 (absmax-relative error ~7e-4, gate is 2e-2).

Hybrid split by channel, balancing DVE against PE (measured HW rates):

 - scan path (CH_SC=512 channels): host stages x[b][:, CH_PE:].T * w[t]
   as [CH_SC, T] fp16 (w[0]=1, w[t]=1/t).  One DVE tensor_tensor_scan per
   128-channel tile yields the running MEAN directly:
       state = (x'[t] + state) * alpha[t],  alpha[t] = t/(t+1) (fp32)
   ~8.7-10.4us per tile (2.1-2.5 ns/elem depending on HAM state).

 - PE path (CH_PE=512 channels): natural [T, CH_PE] layout, 32 row-tiles.
   Phase A: one 32-matmul PSUM accumulation with E-indicator weights
   gives all per-tile column totals; ACT copies them to SBUF.  Carries
   (strict prefix sums of totals) come from a single small Lstrict
   matmul.  They are then folded into ROW 0 of each resident x tile
   (gather-DMA -> one Pool add -> scatter-DMA): since row 0 contributes
   to every cumsum row, the per-tile tril matmul alone then produces the
   full global cumsum — no carry-broadcast matmuls at all.  Phase B is a
   pure tril stream (single weight load), evicted by ACT with the
   per-partition 1/(t+1) scale.

Load order: the first scan's alpha/x chunks head the DMA queue (first
scan starts ~13us), xp batches next so phase A is never load-starved,
remaining scan tiles stream behind with slack.
"""

import sys

import numpy as np

sys.path.insert(0, "/opt/trn_rl_repo")

import concourse.bass as bass
import concourse.mybir as mybir
import concourse.tile as tile
from concourse import bacc
from concourse.bass_utils import run_bass_kernel_spmd

B, T, C = 8, 4096, 1024
P = 128
NT = T // P          # 32 row tiles (PE path)
CH_PE = 512          # channels on the PE path
CH_SC = C - CH_PE    # channels on the scan path
NSC = CH_SC // P     # 4 scan tiles
GB = 4               # row tiles per batched PE DMA
NB = NT // GB        # 8 batches
F16 = mybir.dt.float16
F32 = mybir.dt.float32
ADD = mybir.AluOpType.add
MULT = mybir.AluOpType.mult
COPY = mybir.ActivationFunctionType.Copy

_cache = {}


def _consts():
    tril_t = np.tril(np.ones((P, P), np.float16)).T.copy()
    e_all = np.zeros((P, NT * NT), np.float16)
    for k in range(NT):
        e_all[:, k * NT + k] = 1.0
    # lstrict_t[j, k] = [j < k]  (lhsT of the strict lower-tri ones matrix)
    lstrict_t = np.triu(np.ones((NT, NT), np.float16), 1).copy()
    t_idx = np.arange(T, dtype=np.float64).reshape(NT, P).T  # [P, NT]
    recip = (1.0 / (t_idx + 1.0)).astype(np.float32)
    t = np.arange(T, dtype=np.float64)
    alpha = t / (t + 1.0)
    alpha[0] = 1.0
    alpha_rep = np.ascontiguousarray(
        np.broadcast_to(alpha.astype(np.float32), (P, T))
    )
    w = np.ones(T, dtype=np.float64)
    w[1:] = 1.0 / t[1:]
    return tril_t, e_all, lstrict_t, recip, alpha_rep, w


def _build():
    nc = bacc.Bacc("TRN2", target_bir_lowering=False, debug=False, num_devices=B)
    # xp/ope are staged "p-major" ([P, NT*CH_PE]): partition p's rows for
    # all 32 tiles are contiguous in DRAM -> big contiguous DMA descriptors.
    xp_d = nc.dram_tensor("xp", [P, NT * CH_PE], F16, kind="ExternalInput").ap()
    xs_d = nc.dram_tensor("xs", [CH_SC, T], F16, kind="ExternalInput").ap()
    al_d = nc.dram_tensor("alpha", [P, T], F32, kind="ExternalInput").ap()
    tril_d = nc.dram_tensor("tril_t", [P, P], F16, kind="ExternalInput").ap()
    e_d = nc.dram_tensor("e_all", [P, NT * NT], F16, kind="ExternalInput").ap()
    ls_d = nc.dram_tensor("lstrict_t", [NT, NT], F16, kind="ExternalInput").ap()
    r_d = nc.dram_tensor("recip", [P, NT], F32, kind="ExternalInput").ap()
    ope_d = nc.dram_tensor("ope", [P, NT * CH_PE], F16, kind="ExternalOutput").ap()
    osc_d = nc.dram_tensor("osc", [CH_SC, T], F16, kind="ExternalOutput").ap()

    xp_v = xp_d                                       # [P, NT*CH_PE]
    ope_v = ope_d.rearrange("p (n c) -> p n c", c=CH_PE)
    xs_v = xs_d.rearrange("(n p) t -> n p t", p=P)    # [NSC, P, T]
    osc_v = osc_d.rearrange("(n p) t -> n p t", p=P)

    H = T // 2
    with tile.TileContext(nc) as tc:
        with (
            tc.tile_pool(name="const", bufs=1) as cp,
            tc.tile_pool(name="xres", bufs=1) as xrp,
            tc.tile_pool(name="fix", bufs=1) as fxp,
            tc.tile_pool(name="st", bufs=3) as stp,
            tc.tile_pool(name="sx", bufs=2) as sxp,
            tc.tile_pool(name="so", bufs=2) as sop,
            tc.tile_pool(name="ps", bufs=4, space=bass.MemorySpace.PSUM) as psp,
            tc.tile_pool(name="pt", bufs=2, space=bass.MemorySpace.PSUM) as ptp,
        ):
            al_s = cp.tile([P, T], F32)
            tril_s = cp.tile([P, P], F16)
            e_s = cp.tile([P, NT * NT], F16)
            ls_s = cp.tile([NT, NT], F16)
            r_s = cp.tile([P, NT], F32)
            xr = xrp.tile([P, NT * CH_PE], F16)
            sx_tiles = [
                sxp.tile([P, T], F16, name=f"sx{j}") for j in range(NSC)
            ]

            # ---- DMA queue: scan head, then xp early, xs behind --------
            nc.sync.dma_start(al_s[:, 0:H], al_d[:, 0:H])
            nc.sync.dma_start(sx_tiles[0][:, 0:H], xs_v[0][:, 0:H])
            nc.sync.dma_start(tril_s[:], tril_d)
            nc.sync.dma_start(e_s[:], e_d)
            nc.sync.dma_start(ls_s[:], ls_d)
            nc.sync.dma_start(r_s[:], r_d)
            nc.sync.dma_start(al_s[:, H:T], al_d[:, H:T])
            nc.sync.dma_start(sx_tiles[0][:, H:T], xs_v[0][:, H:T])
            hb = NT * CH_PE // 2
            nc.gpsimd.dma_start(sx_tiles[1][:], xs_v[1])
            nc.sync.dma_start(xr[:, 0:hb], xp_v[:, 0:hb])
            nc.sync.dma_start(xr[:, hb:], xp_v[:, hb:])

            # ---- scan tiles 0 (split in two for an early start) and 1 --
            so_tiles = []
            so0 = sop.tile([P, T], F16, name="so0")
            nc.vector.tensor_tensor_scan(
                so0[:, 0:H], sx_tiles[0][:, 0:H], al_s[:, 0:H], 0.0, ADD, MULT
            )
            nc.vector.tensor_tensor_scan(
                so0[:, H:T], sx_tiles[0][:, H:T], al_s[:, H:T],
                so0[:, H - 1:H], ADD, MULT,
            )
            so_tiles.append(so0)
            nc.gpsimd.dma_start(osc_v[0], so0[:])
            nc.gpsimd.dma_start(sx_tiles[2][:], xs_v[2])
            so1 = sop.tile([P, T], F16, name="so1")
            nc.vector.tensor_tensor_scan(
                so1[:], sx_tiles[1][:], al_s[:], 0.0, ADD, MULT
            )
            so_tiles.append(so1)
            nc.gpsimd.dma_start(osc_v[1], so1[:])
            nc.gpsimd.dma_start(sx_tiles[3][:], xs_v[3])
            for j in range(2, NSC):
                so = sop.tile([P, T], F16, name=f"so{j}")
                nc.vector.tensor_tensor_scan(
                    so[:], sx_tiles[j][:], al_s[:], 0.0, ADD, MULT
                )
                so_tiles.append(so)

            # ---- PE phase A: all 32 totals in one PSUM accumulation ----
            pt = ptp.tile([NT, CH_PE], F32)
            for k in range(NT):
                nc.tensor.matmul(
                    pt[:],
                    e_s[:, k * NT:(k + 1) * NT],
                    xr[:, k * CH_PE:(k + 1) * CH_PE],
                    start=(k == 0),
                    stop=(k == NT - 1),
                )
            tot = fxp.tile([NT, CH_PE], F16)
            nc.scalar.activation(tot[:], pt[:], COPY)

            # ---- carries -> row 0 of each tile (gather/add/scatter) ----
            carr_ps = ptp.tile([NT, CH_PE], F32)
            nc.tensor.matmul(carr_ps[:], ls_s[:], tot[:], start=True, stop=True)
            carr = fxp.tile([NT, CH_PE], F16)
            nc.scalar.activation(carr[:], carr_ps[:], COPY)
            x0g = fxp.tile([NT, CH_PE], F16)
            row0 = xr[0:1, :].rearrange("o (n c) -> o n c", n=NT)  # [1,NT,CH_PE]
            nc.sync.dma_start(x0g[:], row0)
            fixed = fxp.tile([NT, CH_PE], F16)
            nc.gpsimd.tensor_add(fixed[:], x0g[:], carr[:])
            nc.sync.dma_start(row0, fixed[:])
            for j in range(2, NSC):
                nc.gpsimd.dma_start(osc_v[j], so_tiles[j][:])

            # ---- PE phase B: tril stream + scaled eviction + store -----
            st = None
            for k in range(NT):
                ps = psp.tile([P, CH_PE], F32)
                nc.tensor.matmul(
                    ps[:], tril_s[:],
                    xr[:, k * CH_PE:(k + 1) * CH_PE],
                    start=True, stop=True,
                )
                if k % GB == 0:
                    st = stp.tile([P, GB * CH_PE], F16)
                o = st[:, (k % GB) * CH_PE:(k % GB + 1) * CH_PE]
                nc.scalar.activation(o, ps[:], COPY, scale=r_s[:, k:k + 1])
                if k % GB == GB - 1:
                    nc.sync.dma_start(ope_v[:, k - GB + 1:k + 1, :], st[:])

    nc.compile()
    return nc


def _run(x, trace=False):
    x = np.asarray(x)
    assert x.shape == (B, T, C)
    if "nc" not in _cache:
        _cache["consts"] = _consts()
        _cache["nc"] = _build()
    nc = _cache["nc"]
    tril_t, e_all, lstrict_t, recip, alpha_rep, w = _cache["consts"]
    in_maps = []
    for b in range(B):
        xb = x[b]
        xp = np.ascontiguousarray(
            xb[:, :CH_PE].astype(np.float16)
            .reshape(NT, P, CH_PE).transpose(1, 0, 2).reshape(P, NT * CH_PE)
        )
        xs = np.ascontiguousarray(
            (xb[:, CH_PE:].astype(np.float64).T * w[None, :]).astype(np.float16)
        )
        in_maps.append({
            "xp": xp, "xs": xs, "alpha": alpha_rep, "tril_t": tril_t,
            "e_all": e_all, "lstrict_t": lstrict_t, "recip": recip,
        })
    res = run_bass_kernel_spmd(nc, in_maps, core_ids=list(range(B)), trace=trace)
    out = np.empty((B, T, C), np.float32)
    for b in range(B):
        ope = np.asarray(res.results[b]["ope"]).astype(np.float32)
        out[b, :, :CH_PE] = (
            ope.reshape(P, NT, CH_PE).transpose(1, 0, 2).reshape(T, CH_PE)
        )
        out[b, :, CH_PE:] = np.asarray(res.results[b]["osc"]).astype(np.float32).T
    return out, res


def kernel(x):
    out, _ = _run(x, trace=False)
    return out


# revision 35
# speedup vs baseline: 1.1283x; 1.1283x over previous
"""Trainium2 Bass kernel for nn_Lookback: causal running-mean over T.

out[b, t, c] = (1/(t+1)) * sum_{s<=t} x[b, s, c],  x: [8, 4096, 1024] fp32.

Sharding: data-parallel over batch B — core b handles batch b.
All I/O is fp16 (absmax-relative error ~7e-4, gate is 2e-2).

Hybrid split by channel, balancing DVE against PE (measured HW rates):

 - scan path (CH_SC=512 channels): host stages x[b][:, CH_PE:].T * w[t]
   as [CH_SC, T] fp16 (w[0]=1, w[t]=1/t).  One DVE tensor_tensor_scan per
   128-channel tile yields the running MEAN directly:
       state = (x'[t] + state) * alpha[t],  alpha[t] = t/(t+1) (fp32)
   ~8.7-10.4us per tile (2.1-2.5 ns/elem depending on HAM state).

 - PE path (CH_PE=512 channels): natural [T, CH_PE] layout, 32 row-tiles.
   Phase A: one 32-matmul PSUM accumulation with E-indicator weights
   gives all per-tile column totals; ACT copies them to SBUF.  Carries
   (strict prefix sums of totals) come from a single small Lstrict
   matmul.  They are then folded into ROW 0 of each resident x tile
   (gather-DMA -> one Pool add -> scatter-DMA): since row 0 contributes
   to every cumsum row, the per-tile tril matmul alone then produces the
   full global cumsum — no carry-broadcast matmuls at all.  Phase B is a
   pure tril stream (single weight load), evicted by ACT with the
   per-partition 1/(t+1) scale.

Load order: the first scan's alpha/x chunks head the DMA queue (first
scan starts ~13us), xp batches next so phase A is never load-starved,
remaining scan tiles stream behind with slack.
"""

import sys

import numpy as np

sys.path.insert(0, "/opt/trn_rl_repo")

import concourse.bass as bass
import concourse.mybir as mybir
import concourse.tile as tile
from concourse import bacc
from concourse.bass_utils import run_bass_kernel_spmd

B, T, C = 8, 4096, 1024
P = 128
NT = T // P          # 32 row tiles (PE path)
CH_PE = 512          # channels on the PE path
CH_SC = C - CH_PE    # channels on the scan path
NSC = CH_SC // P     # 4 scan tiles
GB = 4               # row tiles per batched PE DMA
NB = NT // GB        # 8 batches
F16 = mybir.dt.float16
F32 = mybir.dt.float32
ADD = mybir.AluOpType.add
MULT = mybir.AluOpType.mult
COPY = mybir.ActivationFunctionType.Copy

_cache = {}


def _consts():
    tril_t = np.tril(np.ones((P, P), np.float16)).T.copy()
    e_all = np.zeros((P, NT * NT), np.float16)
    for k in range(NT):
        e_all[:, k * NT + k] = 1.0
    # lstrict_t[j, k] = [j < k]  (lhsT of the strict lower-tri ones matrix)
    lstrict_t = np.triu(np.ones((NT, NT), np.float16), 1).copy()
    t_idx = np.arange(T, dtype=np.float64).reshape(NT, P).T  # [P, NT]
    recip = (1.0 / (t_idx + 1.0)).astype(np.float32)
    t = np.arange(T, dtype=np.float64)
    alpha = t / (t + 1.0)
    alpha[0] = 1.0
    alpha_rep = np.ascontiguousarray(
        np.broadcast_to(alpha.astype(np.float32), (P, T))
    )
    w = np.ones(T, dtype=np.float64)
    w[1:] = 1.0 / t[1:]
    return tril_t, e_all, lstrict_t, recip, alpha_rep, w


def _build():
    nc = bacc.Bacc("TRN2", target_bir_lowering=False, debug=False, num_devices=B)
    # xp/ope staged "p-major" ([P, NT*CH_PE]): partition p's rows for all
    # 32 tiles are contiguous in DRAM -> multi-KB DMA descriptors.
    xp_d = nc.dram_tensor("xp", [P, NT * CH_PE], F16, kind="ExternalInput").ap()
    xs_d = nc.dram_tensor("xs", [CH_SC, T], F16, kind="ExternalInput").ap()
    al_d = nc.dram_tensor("alpha", [P, T], F32, kind="ExternalInput").ap()
    tril_d = nc.dram_tensor("tril_t", [P, P], F16, kind="ExternalInput").ap()
    e_d = nc.dram_tensor("e_all", [P, NT * NT], F16, kind="ExternalInput").ap()
    ls_d = nc.dram_tensor("lstrict_t", [NT, NT], F16, kind="ExternalInput").ap()
    r_d = nc.dram_tensor("recip", [P, NT], F32, kind="ExternalInput").ap()
    ope_d = nc.dram_tensor("ope", [P, NT * CH_PE], F16, kind="ExternalOutput").ap()
    osc_d = nc.dram_tensor("osc", [CH_SC, T], F16, kind="ExternalOutput").ap()

    xp_v = xp_d                                       # [P, NT*CH_PE]
    ope_v = ope_d.rearrange("p (n c) -> p n c", c=CH_PE)
    xs_v = xs_d.rearrange("(n p) t -> n p t", p=P)    # [NSC, P, T]
    osc_v = osc_d.rearrange("(n p) t -> n p t", p=P)

    H = T // 2
    with tile.TileContext(nc) as tc:
        with (
            tc.tile_pool(name="const", bufs=1) as cp,
            tc.tile_pool(name="xres", bufs=1) as xrp,
            tc.tile_pool(name="fix", bufs=1) as fxp,
            tc.tile_pool(name="st", bufs=3) as stp,
            tc.tile_pool(name="sx", bufs=2) as sxp,
            tc.tile_pool(name="so", bufs=2) as sop,
            tc.tile_pool(name="ps", bufs=4, space=bass.MemorySpace.PSUM) as psp,
            tc.tile_pool(name="pt", bufs=2, space=bass.MemorySpace.PSUM) as ptp,
        ):
            al_s = cp.tile([P, T], F32)
            tril_s = cp.tile([P, P], F16)
            e_s = cp.tile([P, NT * NT], F16)
            ls_s = cp.tile([NT, NT], F16)
            r_s = cp.tile([P, NT], F32)
            xr = xrp.tile([P, NT * CH_PE], F16)
            sx_tiles = [
                sxp.tile([P, T], F16, name=f"sx{j}") for j in range(NSC)
            ]

            # ---- DMA queue: scan head, then xp early, xs behind --------
            nc.sync.dma_start(al_s[:, 0:H], al_d[:, 0:H])
            nc.sync.dma_start(sx_tiles[0][:, 0:H], xs_v[0][:, 0:H])
            nc.sync.dma_start(tril_s[:], tril_d)
            nc.sync.dma_start(e_s[:], e_d)
            nc.sync.dma_start(ls_s[:], ls_d)
            nc.sync.dma_start(r_s[:], r_d)
            nc.sync.dma_start(al_s[:, H:T], al_d[:, H:T])
            nc.sync.dma_start(sx_tiles[0][:, H:T], xs_v[0][:, H:T])
            hb = NT * CH_PE // 2
            nc.sync.dma_start(xr[:, 0:hb], xp_v[:, 0:hb])
            nc.sync.dma_start(sx_tiles[1][:], xs_v[1])
            nc.sync.dma_start(xr[:, hb:], xp_v[:, hb:])

            # ---- scan tiles 0 (split in two for an early start) and 1 --
            so_tiles = []
            so0 = sop.tile([P, T], F16, name="so0")
            nc.vector.tensor_tensor_scan(
                so0[:, 0:H], sx_tiles[0][:, 0:H], al_s[:, 0:H], 0.0, ADD, MULT
            )
            nc.vector.tensor_tensor_scan(
                so0[:, H:T], sx_tiles[0][:, H:T], al_s[:, H:T],
                so0[:, H - 1:H], ADD, MULT,
            )
            so_tiles.append(so0)
            so1 = sop.tile([P, T], F16, name="so1")
            nc.vector.tensor_tensor_scan(
                so1[:], sx_tiles[1][:], al_s[:], 0.0, ADD, MULT
            )
            so_tiles.append(so1)

            # ---- PE phase A: all 32 totals in one PSUM accumulation ----
            pt = ptp.tile([NT, CH_PE], F32)
            for k in range(NT):
                nc.tensor.matmul(
                    pt[:],
                    e_s[:, k * NT:(k + 1) * NT],
                    xr[:, k * CH_PE:(k + 1) * CH_PE],
                    start=(k == 0),
                    stop=(k == NT - 1),
                )
            tot = fxp.tile([NT, CH_PE], F16)
            nc.scalar.activation(tot[:], pt[:], COPY)

            # ---- carries -> row 0 of each tile (gather/add/scatter) ----
            carr_ps = ptp.tile([NT, CH_PE], F32)
            nc.tensor.matmul(carr_ps[:], ls_s[:], tot[:], start=True, stop=True)
            carr = fxp.tile([NT, CH_PE], F16)
            nc.scalar.activation(carr[:], carr_ps[:], COPY)
            x0g = fxp.tile([NT, CH_PE], F16)
            row0 = xr[0:1, :].rearrange("o (n c) -> o n c", n=NT)  # [1,NT,CH_PE]
            nc.sync.dma_start(x0g[:], row0)
            fixed = fxp.tile([NT, CH_PE], F16)
            nc.gpsimd.tensor_add(fixed[:], x0g[:], carr[:])
            nc.sync.dma_start(row0, fixed[:])

            # ---- PE phase B: tril stream + scaled eviction + store -----
            # SP-queue order tracks completion order: scan outs / late
            # scan loads+scans / PE output batches interleaved by readiness.
            sp_plan = {0: [("out", 0), ("load", 2)], 1: [("out", 1), ("load", 3)],
                       3: [("out", 2)], 6: [("out", 3)]}
            st = None
            for k in range(NT):
                ps = psp.tile([P, CH_PE], F32)
                nc.tensor.matmul(
                    ps[:], tril_s[:],
                    xr[:, k * CH_PE:(k + 1) * CH_PE],
                    start=True, stop=True,
                )
                if k % GB == 0:
                    st = stp.tile([P, GB * CH_PE], F16)
                o = st[:, (k % GB) * CH_PE:(k % GB + 1) * CH_PE]
                nc.scalar.activation(o, ps[:], COPY, scale=r_s[:, k:k + 1])
                if k % GB == GB - 1:
                    m = k // GB
                    nc.sync.dma_start(ope_v[:, k - GB + 1:k + 1, :], st[:])
                    for kind, j in sp_plan.get(m, []):
                        if kind == "out":
                            nc.sync.dma_start(osc_v[j], so_tiles[j][:])
                        elif j < NSC:
                            nc.sync.dma_start(sx_tiles[j][:], xs_v[j])
                            so = sop.tile([P, T], F16, name=f"so{j}")
                            nc.vector.tensor_tensor_scan(
                                so[:], sx_tiles[j][:], al_s[:], 0.0, ADD, MULT
                            )
                            so_tiles.append(so)

    nc.compile()
    return nc


def _run(x, trace=False):
    x = np.asarray(x)
    assert x.shape == (B, T, C)
    if "nc" not in _cache:
        _cache["consts"] = _consts()
        _cache["nc"] = _build()
    nc = _cache["nc"]
    tril_t, e_all, lstrict_t, recip, alpha_rep, w = _cache["consts"]
    in_maps = []
    for b in range(B):
        xb = x[b]
        xp = np.ascontiguousarray(
            xb[:, :CH_PE].astype(np.float16)
            .reshape(NT, P, CH_PE).transpose(1, 0, 2).reshape(P, NT * CH_PE)
        )
        xs = np.ascontiguousarray(
            (xb[:, CH_PE:].astype(np.float64).T * w[None, :]).astype(np.float16)
        )
        in_maps.append({
            "xp": xp, "xs": xs, "alpha": alpha_rep, "tril_t": tril_t,
            "e_all": e_all, "lstrict_t": lstrict_t, "recip": recip,
        })
    res = run_bass_kernel_spmd(nc, in_maps, core_ids=list(range(B)), trace=trace)
    out = np.empty((B, T, C), np.float32)
    for b in range(B):
        ope = np.asarray(res.results[b]["ope"]).astype(np.float32)
        out[b, :, :CH_PE] = (
            ope.reshape(P, NT, CH_PE).transpose(1, 0, 2).reshape(T, CH_PE)
        )
        out[b, :, CH_PE:] = np.asarray(res.results[b]["osc"]).astype(np.float32).T
    return out, res


def kernel(x):
    out, _ = _run(x, trace=False)
    return out


# revision 36
# speedup vs baseline: 1.2395x; 1.0986x over previous
"""Trainium2 Bass kernel for nn_Lookback: causal running-mean over T.

out[b, t, c] = (1/(t+1)) * sum_{s<=t} x[b, s, c],  x: [8, 4096, 1024] fp32.

Sharding: data-parallel over batch B — core b handles batch b.
All I/O is fp16 (absmax-relative error ~7e-4, gate is 2e-2).

Hybrid split by channel, balancing DVE against PE (measured HW rates):

 - scan path (CH_SC=512 channels): host stages x[b][:, CH_PE:].T * w[t]
   as [CH_SC, T] fp16 (w[0]=1, w[t]=1/t).  One DVE tensor_tensor_scan per
   128-channel tile yields the running MEAN directly:
       state = (x'[t] + state) * alpha[t],  alpha[t] = t/(t+1) (fp32)
   ~8.7-10.4us per tile (2.1-2.5 ns/elem depending on HAM state).

 - PE path (CH_PE=512 channels): natural [T, CH_PE] layout, 32 row-tiles.
   Phase A: one 32-matmul PSUM accumulation with E-indicator weights
   gives all per-tile column totals; ACT copies them to SBUF.  Carries
   (strict prefix sums of totals) come from a single small Lstrict
   matmul.  They are then folded into ROW 0 of each resident x tile
   (gather-DMA -> one Pool add -> scatter-DMA): since row 0 contributes
   to every cumsum row, the per-tile tril matmul alone then produces the
   full global cumsum — no carry-broadcast matmuls at all.  Phase B is a
   pure tril stream (single weight load), evicted by ACT with the
   per-partition 1/(t+1) scale.

Load order: the first scan's alpha/x chunks head the DMA queue (first
scan starts ~13us), xp batches next so phase A is never load-starved,
remaining scan tiles stream behind with slack.
"""

import sys

import numpy as np

sys.path.insert(0, "/opt/trn_rl_repo")

import concourse.bass as bass
import concourse.mybir as mybir
import concourse.tile as tile
from concourse import bacc
from concourse.bass_utils import run_bass_kernel_spmd

B, T, C = 8, 4096, 1024
P = 128
NT = T // P          # 32 row tiles (PE path)
CH_PE = 512          # channels on the PE path
CH_SC = C - CH_PE    # channels on the scan path
NSC = CH_SC // P     # 4 scan tiles
GB = 4               # row tiles per batched PE DMA
NB = NT // GB        # 8 batches
F16 = mybir.dt.float16
F32 = mybir.dt.float32
ADD = mybir.AluOpType.add
MULT = mybir.AluOpType.mult
COPY = mybir.ActivationFunctionType.Copy

_cache = {}


def _consts():
    tril_t = np.tril(np.ones((P, P), np.float16)).T.copy()
    e_all = np.zeros((P, NT * NT), np.float16)
    for k in range(NT):
        e_all[:, k * NT + k] = 1.0
    # lstrict_t[j, k] = [j < k]  (lhsT of the strict lower-tri ones matrix)
    lstrict_t = np.triu(np.ones((NT, NT), np.float16), 1).copy()
    t_idx = np.arange(T, dtype=np.float64).reshape(NT, P).T  # [P, NT]
    recip = (1.0 / (t_idx + 1.0)).astype(np.float32)
    t = np.arange(T, dtype=np.float64)
    alpha = t / (t + 1.0)
    alpha[0] = 1.0
    alpha_rep = np.ascontiguousarray(
        np.broadcast_to(alpha.astype(np.float32), (P, T))
    )
    w = np.ones(T, dtype=np.float64)
    w[1:] = 1.0 / t[1:]
    return tril_t, e_all, lstrict_t, recip, alpha_rep, w


def _build():
    nc = bacc.Bacc("TRN2", target_bir_lowering=False, debug=False, num_devices=B)
    xp_d = nc.dram_tensor("xp", [T, CH_PE], F16, kind="ExternalInput").ap()
    xs_d = nc.dram_tensor("xs", [CH_SC, T], F16, kind="ExternalInput").ap()
    al_d = nc.dram_tensor("alpha", [P, T], F32, kind="ExternalInput").ap()
    tril_d = nc.dram_tensor("tril_t", [P, P], F16, kind="ExternalInput").ap()
    e_d = nc.dram_tensor("e_all", [P, NT * NT], F16, kind="ExternalInput").ap()
    ls_d = nc.dram_tensor("lstrict_t", [NT, NT], F16, kind="ExternalInput").ap()
    r_d = nc.dram_tensor("recip", [P, NT], F32, kind="ExternalInput").ap()
    ope_d = nc.dram_tensor("ope", [T, CH_PE], F16, kind="ExternalOutput").ap()
    osc_d = nc.dram_tensor("osc", [CH_SC, T], F16, kind="ExternalOutput").ap()

    xp_v = xp_d.rearrange("(n p) c -> p n c", p=P)    # [P, NT, CH_PE]
    ope_v = ope_d.rearrange("(n p) c -> p n c", p=P)
    xs_v = xs_d.rearrange("(n p) t -> n p t", p=P)    # [NSC, P, T]
    osc_v = osc_d.rearrange("(n p) t -> n p t", p=P)

    H = T // 2
    with tile.TileContext(nc) as tc:
        with (
            tc.tile_pool(name="const", bufs=1) as cp,
            tc.tile_pool(name="xres", bufs=1) as xrp,
            tc.tile_pool(name="fix", bufs=1) as fxp,
            tc.tile_pool(name="st", bufs=3) as stp,
            tc.tile_pool(name="sx", bufs=2) as sxp,
            tc.tile_pool(name="so", bufs=2) as sop,
            tc.tile_pool(name="ps", bufs=4, space=bass.MemorySpace.PSUM) as psp,
            tc.tile_pool(name="pt", bufs=2, space=bass.MemorySpace.PSUM) as ptp,
        ):
            al_s = cp.tile([P, T], F32)
            tril_s = cp.tile([P, P], F16)
            e_s = cp.tile([P, NT * NT], F16)
            ls_s = cp.tile([NT, NT], F16)
            r_s = cp.tile([P, NT], F32)
            xr = xrp.tile([P, NT * CH_PE], F16)
            sx_tiles = [
                sxp.tile([P, T], F16, name=f"sx{j}") for j in range(NSC)
            ]

            # ---- DMA queue: scan head, then xp early, xs behind --------
            nc.sync.dma_start(al_s[:, 0:H], al_d[:, 0:H])
            nc.sync.dma_start(sx_tiles[0][:, 0:H], xs_v[0][:, 0:H])
            nc.sync.dma_start(tril_s[:], tril_d)
            nc.sync.dma_start(e_s[:], e_d)
            nc.sync.dma_start(ls_s[:], ls_d)
            nc.sync.dma_start(r_s[:], r_d)
            nc.sync.dma_start(al_s[:, H:T], al_d[:, H:T])
            nc.sync.dma_start(sx_tiles[0][:, H:T], xs_v[0][:, H:T])
            for m in range(NB):
                nc.sync.dma_start(
                    xr[:, m * GB * CH_PE:(m + 1) * GB * CH_PE],
                    xp_v[:, m * GB:(m + 1) * GB, :],
                )
                if m == 3:
                    nc.sync.dma_start(sx_tiles[1][:], xs_v[1])

            # ---- scan tiles 0 (split in two for an early start) and 1 --
            so_tiles = []
            so0 = sop.tile([P, T], F16, name="so0")
            nc.vector.tensor_tensor_scan(
                so0[:, 0:H], sx_tiles[0][:, 0:H], al_s[:, 0:H], 0.0, ADD, MULT
            )
            nc.vector.tensor_tensor_scan(
                so0[:, H:T], sx_tiles[0][:, H:T], al_s[:, H:T],
                so0[:, H - 1:H], ADD, MULT,
            )
            so_tiles.append(so0)
            so1 = sop.tile([P, T], F16, name="so1")
            nc.vector.tensor_tensor_scan(
                so1[:], sx_tiles[1][:], al_s[:], 0.0, ADD, MULT
            )
            so_tiles.append(so1)

            # ---- PE phase A: all 32 totals in one PSUM accumulation ----
            pt = ptp.tile([NT, CH_PE], F32)
            for k in range(NT):
                nc.tensor.matmul(
                    pt[:],
                    e_s[:, k * NT:(k + 1) * NT],
                    xr[:, k * CH_PE:(k + 1) * CH_PE],
                    start=(k == 0),
                    stop=(k == NT - 1),
                )
            tot = fxp.tile([NT, CH_PE], F16)
            nc.scalar.activation(tot[:], pt[:], COPY)

            # ---- carries -> row 0 of each tile (gather/add/scatter) ----
            carr_ps = ptp.tile([NT, CH_PE], F32)
            nc.tensor.matmul(carr_ps[:], ls_s[:], tot[:], start=True, stop=True)
            carr = fxp.tile([NT, CH_PE], F16)
            nc.scalar.activation(carr[:], carr_ps[:], COPY)
            x0g = fxp.tile([NT, CH_PE], F16)
            row0 = xr[0:1, :].rearrange("o (n c) -> o n c", n=NT)  # [1,NT,CH_PE]
            nc.sync.dma_start(x0g[:], row0)
            fixed = fxp.tile([NT, CH_PE], F16)
            nc.gpsimd.tensor_add(fixed[:], x0g[:], carr[:])
            nc.sync.dma_start(row0, fixed[:])

            # ---- PE phase B: tril stream + scaled eviction + store -----
            # SP-queue order tracks completion order: scan outs / late
            # scan loads+scans / PE output batches interleaved by readiness.
            sp_plan = {0: [("out", 0), ("load", 2)], 1: [("out", 1), ("load", 3)],
                       3: [("out", 2)], 6: [("out", 3)]}
            st = None
            for k in range(NT):
                ps = psp.tile([P, CH_PE], F32)
                nc.tensor.matmul(
                    ps[:], tril_s[:],
                    xr[:, k * CH_PE:(k + 1) * CH_PE],
                    start=True, stop=True,
                )
                if k % GB == 0:
                    st = stp.tile([P, GB * CH_PE], F16)
                o = st[:, (k % GB) * CH_PE:(k % GB + 1) * CH_PE]
                nc.scalar.activation(o, ps[:], COPY, scale=r_s[:, k:k + 1])
                if k % GB == GB - 1:
                    m = k // GB
                    nc.sync.dma_start(ope_v[:, k - GB + 1:k + 1, :], st[:])
                    for kind, j in sp_plan.get(m, []):
                        if kind == "out":
                            nc.sync.dma_start(osc_v[j], so_tiles[j][:])
                        elif j < NSC:
                            nc.sync.dma_start(sx_tiles[j][:], xs_v[j])
                            so = sop.tile([P, T], F16, name=f"so{j}")
                            nc.vector.tensor_tensor_scan(
                                so[:], sx_tiles[j][:], al_s[:], 0.0, ADD, MULT
                            )
                            so_tiles.append(so)

    nc.compile()
    return nc


def _run(x, trace=False):
    x = np.asarray(x)
    assert x.shape == (B, T, C)
    if "nc" not in _cache:
        _cache["consts"] = _consts()
        _cache["nc"] = _build()
    nc = _cache["nc"]
    tril_t, e_all, lstrict_t, recip, alpha_rep, w = _cache["consts"]
    in_maps = []
    for b in range(B):
        xb = x[b]
        xp = np.ascontiguousarray(xb[:, :CH_PE].astype(np.float16))
        xs = np.ascontiguousarray(
            (xb[:, CH_PE:].astype(np.float64).T * w[None, :]).astype(np.float16)
        )
        in_maps.append({
            "xp": xp, "xs": xs, "alpha": alpha_rep, "tril_t": tril_t,
            "e_all": e_all, "lstrict_t": lstrict_t, "recip": recip,
        })
    res = run_bass_kernel_spmd(nc, in_maps, core_ids=list(range(B)), trace=trace)
    out = np.empty((B, T, C), np.float32)
    for b in range(B):
        out[b, :, :CH_PE] = np.asarray(res.results[b]["ope"]).astype(np.float32)
        out[b, :, CH_PE:] = np.asarray(res.results[b]["osc"]).astype(np.float32).T
    return out, res


def kernel(x):
    out, _ = _run(x, trace=False)
    return out
